# revision 55
# baseline (speedup 1.0000x reference)
"""MLA (multi-head latent attention) forward kernel for Trainium2, 8 NeuronCores.

Sharding: 8 cores = 2 (batch) x 4 (head-groups of 10 heads).
Each core computes, for its batch b and its 10 heads:
  - 1/4 of the fused down-projection a = x @ w_a (sequence-sharded within the
    batch group, transposed-activation layout), rmsnorm + k_pe rope on its
    slice, then AllGathers of the normalized latents across the 4 cores
  - q/kv up-projections for its heads, causal attention, and the partial
    o-projection (w_o rows of its heads).  Host sums the 4 partials per batch.

Schedule/overlap design (v2):
  - the kv latents gather (bf16, ~2.6MB out) is issued right after the 5
    kv/pe m-tiles of phase A and completes under the q m-tiles.
  - the q latents are normalized into fp8(e3m4) and gathered in TWO halves
    (by local token range) so the collectives pipeline under phase B and the
    first half of phase C; phase C consumes 256-token sub-chunks.
  - all DMAs that depend on a collective are issued from the Activation
    queue with tile_wait_until hints so they never head-of-line-block the
    SP queue (which streams the weight tiles).
  - phase D trims the causal diagonal at 128-query granularity and fuses
    exp over pairs of key tiles ([128,1024] PSUM) to offload the Act engine.
  - a short warm-up matmul chain runs during the initial x load so the PE
    p-state is at full clock when real work starts.

Device layout notes:
  - activations are kept transposed ([feature, seq]) so weights act as the
    stationary lhsT operand of the PE in their natural [in, out] orientation.
  - attention computes scoresT [keys, q]; softmax runs without max-subtraction
    (scores are bounded by construction), masking is a binary multiply on the
    exp'd probabilities, and sum-of-exp comes from a ones-column appended to V
    in the AV matmul.  Per-row 1/sum is applied on PSUM eviction.
  - qT round-trips through DRAM so SBUF tile-pool lifetimes nest properly.
"""

import math
import sys
from dataclasses import dataclass

if "/opt/trn_rl_repo" not in sys.path:
    sys.path.insert(0, "/opt/trn_rl_repo")

import ml_dtypes
import numpy as np

BF16 = ml_dtypes.bfloat16


@dataclass(frozen=True)
class Cfg:
    HID: int = 5120
    S: int = 2048
    QLR: int = 1536
    KVLR: int = 512
    DN: int = 128
    DR: int = 64
    DV: int = 128
    HPC: int = 10          # heads per core
    CHUNK: int = 512       # q-position chunk (PSUM bank width)
    GS: int = 1            # cores per batch group (sequence-shard of phase A)
    NCORES: int = 8
    EPS: float = 1e-6
    THETA: float = 10000.0

    @property
    def DQK(self):
        return self.DN + self.DR

    @property
    def PEH(self):
        return self.DR // 2

    @property
    def SL(self):
        return self.S // self.GS


FULL = Cfg(GS=4)


def build_program(c: Cfg, stop_after: str = "E"):
    import contextlib

    import concourse.bass as bass  # noqa: F401
    import concourse.mybir as mybir
    import concourse.tile as tile
    from concourse import bacc
    from concourse.masks import make_identity

    dt = mybir.dt
    BF = dt.bfloat16
    F32 = dt.float32
    FP8 = dt.float8e3
    Alu = mybir.AluOpType
    Act = mybir.ActivationFunctionType

    KT_HID = c.HID // 128
    KT_Q = c.QLR // 128
    KT_KV = c.KVLR // 128
    NQC = c.S // c.CHUNK
    GS = c.GS
    SL = c.SL
    NLC = SL // c.CHUNK             # local q-chunks in phase A
    HCH = SL // 2                   # local token half for the q gathers
    ST = c.S // 128
    H = c.HPC
    TPC = c.CHUNK // 128            # 128-tiles per chunk (4)
    QROWS = H * (c.DN + c.DR)
    MT_QN = H * c.DN // 128
    MT_QP = H * c.DR // 128
    MT_QB = MT_QN + MT_QP
    KROWS = H * c.DN
    VCOLS = H * c.DV
    MT_O = c.HID // 128
    SCALE = 1.0 / math.sqrt(c.DQK)
    WSC = 32.0           # host-side w_qb up-scale (fp8 subnormal avoidance)
    LSC = 1.0 / 16.0     # latent down-scale; qTs carries q * WSC*LSC = 2x
    ESC = SCALE / (WSC * LSC)

    assert c.DN == 128 and c.DV == 128 and c.DR == 64 and H % 2 == 0
    assert SL % c.CHUNK == 0
    _PH = ["A", "B", "C", "D", "E"]
    enabled = set(_PH[:_PH.index(stop_after) + 1])

    nc = bacc.Bacc("TRN2", num_devices=(c.NCORES if GS > 1 else None))
    MT_A = KT_Q + KT_KV + 1
    xT = nc.dram_tensor("xT", [128, KT_HID * SL], BF, kind="ExternalInput")
    w_a = nc.dram_tensor("w_a", [128, MT_A * KT_HID * 128], BF,
                         kind="ExternalInput")
    w_qb = nc.dram_tensor("w_qb", [128, MT_QB * KT_Q * 128], FP8,
                          kind="ExternalInput")
    w_kvb = nc.dram_tensor("w_kvb", [128, KT_KV * (KROWS + VCOLS)], BF,
                           kind="ExternalInput")
    w_o = nc.dram_tensor("w_o", [128, MT_O * H * 128], BF,
                         kind="ExternalInput")
    cosT = nc.dram_tensor("cosT", [128, c.S], BF, kind="ExternalInput")
    sinT = nc.dram_tensor("sinT", [128, c.S], BF, kind="ExternalInput")
    cosA = nc.dram_tensor("cosA", [128, SL], BF, kind="ExternalInput")
    sinA = nc.dram_tensor("sinA", [128, SL], BF, kind="ExternalInput")
    lnkv = nc.dram_tensor("lnkv", [128, KT_KV], F32, kind="ExternalInput")
    maskm = nc.dram_tensor("maskm", [128, TPC, c.CHUNK], FP8,
                           kind="ExternalInput")
    outT = nc.dram_tensor("outT", [c.HID, c.S], F32, kind="ExternalOutput")
    qTs = nc.dram_tensor("qTs", [QROWS, c.S], FP8, kind="Internal")
    aglkv = nc.dram_tensor("aglkv", [(KT_KV + 1) * 128, SL], BF, kind="Internal")
    aglq0 = nc.dram_tensor("aglq0", [(KT_Q + 1) * 128, HCH], FP8,
                           kind="Internal")
    aglq1 = nc.dram_tensor("aglq1", [(KT_Q + 1) * 128, HCH], FP8,
                           kind="Internal")
    if GS > 1:
        aggkv = nc.dram_tensor("aggkv", [GS * (KT_KV + 1) * 128, SL], BF,
                               kind="Internal")
        aggq0 = nc.dram_tensor("aggq0", [GS * (KT_Q + 1) * 128, HCH], FP8,
                               kind="Internal")
        aggq1 = nc.dram_tensor("aggq1", [GS * (KT_Q + 1) * 128, HCH], FP8,
                               kind="Internal")
    else:
        aggkv = aglkv
        aggq0 = aglq0
        aggq1 = aglq1

    xT_r = xT.ap().rearrange("p (t s) -> p t s", s=SL)
    w_a_r = w_a.ap().rearrange("p (mt k m) -> p mt (k m)", mt=MT_A, m=128)
    w_qb_r = w_qb.ap().rearrange("p (mt k m) -> p mt k m", mt=MT_QB, m=128)
    w_kvb_r = w_kvb.ap().rearrange("p (k m) -> p k m", k=KT_KV)
    w_o_r = w_o.ap().rearrange("p (mt k m) -> p mt (k m)", mt=MT_O, m=128)
    aglkv_r = aglkv.ap().rearrange("(t p) s -> p t s", p=128)
    aglq0_r = aglq0.ap().rearrange("(t p) s -> p t s", p=128)
    aglq1_r = aglq1.ap().rearrange("(t p) s -> p t s", p=128)
    aggkv_r = aggkv.ap().rearrange("(g t p) s -> g p t s", g=GS, p=128)
    aggq0_r = aggq0.ap().rearrange("(g t p) s -> g p t s", g=GS, p=128)
    aggq1_r = aggq1.ap().rearrange("(g t p) s -> g p t s", g=GS, p=128)
    qTs_ap = qTs.ap()
    outT_ap = outT.ap()

    def emit_rope(nc, pool, dst64, src64, cos_ap, sin_ap, W, p0=0, eng=None):
        # cos_ap/sin_ap are [128, W] (table replicated every PEH partitions);
        # slices are taken at each operand's base partition because the
        # vector engines require equal base partitions for SBUF inputs.
        eng = eng or nc.vector
        ph = c.PEH
        t1, t2 = src64[0:ph], src64[ph:2 * ph]
        d1, d2 = dst64[0:ph], dst64[ph:2 * ph]
        c1, s1 = cos_ap[p0:p0 + ph], sin_ap[p0:p0 + ph]
        c2, s2 = cos_ap[p0 + ph:p0 + 2 * ph], sin_ap[p0 + ph:p0 + 2 * ph]
        ra = pool.tile([ph, W], BF, tag="rope_a", bufs=2, name="rope_a")
        rb = pool.tile([ph, W], BF, tag="rope_b", bufs=2, name="rope_b")
        eng.tensor_tensor(out=ra, in0=t1, in1=c1, op=Alu.mult)
        eng.tensor_tensor(out=rb, in0=t2, in1=s2, op=Alu.mult)
        eng.tensor_tensor(out=d1, in0=ra, in1=rb, op=Alu.subtract)
        eng.tensor_tensor(out=ra, in0=t2, in1=c2, op=Alu.mult)
        eng.tensor_tensor(out=rb, in0=t1, in1=s1, op=Alu.mult)
        eng.tensor_tensor(out=d2, in0=ra, in1=rb, op=Alu.add)

    with tile.TileContext(nc, pool_alloc_mode="queue") as tc:
        with contextlib.ExitStack() as top:
            pers = top.enter_context(tc.tile_pool(name="pers", bufs=1))
            lnkv_sb = pers.tile([128, KT_KV], F32, tag="lnkv_sb")
            ident = pers.tile([128, 128], BF, tag="ident")
            ones_fb = pers.tile([1, 128], BF, tag="ones_fb")
            ones_c = pers.tile([128, 1], BF, tag="ones_c")
            eps_sb = pers.tile([1, 1], F32, tag="eps_sb")
            nc.vector.memset(eps_sb, c.EPS)
            kpe = pers.tile([c.DR, c.S], BF, tag="kpe")
            make_identity(nc, ident)
            lnsc_sb = pers.tile([1, 1], F32, tag="lnsc_sb")
            nc.vector.memset(ones_c, 1.0)
            nc.vector.memset(lnsc_sb, math.log(LSC))

            # PE p-state warm-up while the first x/w tiles stream in, so real
            # work starts at (nearly) full clock.  No DMA dependencies.
            with tc.tile_pool(name="pwu", bufs=1, space="PSUM") as pwu:
                wu = pwu.tile([128, 128], F32, tag="wu")
                for _ in range(190):
                    nc.tensor.matmul(wu, ident, ident, start=True, stop=True)

            # gather-landing pool: created before phase A so the reload DMAs
            # can be emitted inside phase A, pinning their gpsimd-queue order
            # between the kv and q collectives.
            pga = top.enter_context(tc.tile_pool(name="pga", bufs=1))
            akv_f = pga.tile([128, KT_KV, c.S], BF, tag="akv_f")

            # -------- phase A: local a-proj + rmsnorm + k_pe rope + gather ---
            with contextlib.ExitStack() as st:
                pax = st.enter_context(tc.tile_pool(name="pax", bufs=1))
                paw = st.enter_context(tc.tile_pool(name="paw", bufs=2))
                pat = st.enter_context(tc.tile_pool(name="pat", bufs=2))
                paa = st.enter_context(tc.tile_pool(name="paa", bufs=1))
                paps = st.enter_context(
                    tc.tile_pool(name="paps", bufs=3, space="PSUM"))
                pssq = st.enter_context(
                    tc.tile_pool(name="pssq", bufs=1, space="PSUM"))
                pbc = st.enter_context(
                    tc.tile_pool(name="pbc", bufs=2, space="PSUM"))

                mtiles = ([("kv", i) for i in range(KT_KV)]
                          + [("pe", 0)]
                          + [("q", i) for i in range(KT_Q)])
                groups = [[b * GS + j for j in range(GS)]
                          for b in range(c.NCORES // GS)]
                # x streamed in 4 k-tile chunks; the small persistent tables
                # are loaded after the first chunks so the first m-tile's
                # operands win the (serialized) DMA bandwidth early.
                x_sb = pax.tile([128, KT_HID, SL], BF, tag="x_sb")
                cosa_sb = pax.tile([128, SL], BF, tag="cosa_sb")
                sina_sb = pax.tile([128, SL], BF, tag="sina_sb")
                ones_f = pax.tile([1, 128], F32, tag="ones_f")
                nc.vector.memset(ones_f, 1.0)
                XC = KT_HID // 4
                for xc in range(2):
                    nc.sync.dma_start(out=x_sb[:, xc * XC:(xc + 1) * XC],
                                      in_=xT_r[:, xc * XC:(xc + 1) * XC])
                aq8 = paa.tile([128, KT_Q, SL], FP8, tag="aq8")
                aq8r = paa.tile([1, 2, HCH], FP8, tag="aq8r")
                akv_c = paa.tile([128, KT_KV, SL], BF, tag="akv_c")
                ssq_q = pssq.tile([1, SL], F32, tag="ssq_q")
                ssq_kv = pssq.tile([1, SL], F32, tag="ssq_kv")
                kperaw = pat.tile([c.DR, SL], BF, tag="kperaw", bufs=1)
                kpel = pat.tile([c.DR, SL], BF, tag="kpel", bufs=1)

                def normalize(ssq, ln_sb, ktn, denom, src, dst):
                    for qi in range(NLC):
                        cs = slice(qi * c.CHUNK, (qi + 1) * c.CHUNK)
                        rn = pat.tile([1, c.CHUNK], F32, tag="rn", bufs=1,
                                      name="rn")
                        nc.scalar.activation(
                            rn, ssq[:, cs], Act.Sqrt,
                            bias=eps_sb, scale=1.0 / denom)
                        rnr = pat.tile([1, c.CHUNK], F32, tag="rnr", bufs=1,
                                       name="rnr")
                        nc.vector.reciprocal(rnr, rn)
                        bc = pbc.tile([128, c.CHUNK], F32, tag="bc", name="bc")
                        nc.tensor.matmul(
                            bc, ones_f, rnr, start=True, stop=True)
                        for t in range(ktn):
                            nc.vector.scalar_tensor_tensor(
                                out=dst[:, t, cs], in0=src[:, t, cs],
                                scalar=ln_sb[:, t:t + 1], in1=bc,
                                op0=Alu.mult, op1=Alu.mult)

                def emit_kv_part():
                    # normalize kv + rope k_pe, spill, and gather — emitted
                    # before the q m-tiles so the collective overlaps them.
                    # All collective-chain DMAs ride the (otherwise idle)
                    # gpsimd queue so their long sem-waits never head-of-line
                    # block the SP/Act streams.
                    normalize(ssq_kv, lnkv_sb, KT_KV, c.KVLR, akv_c, akv_c)
                    for qi in range(NLC):
                        cs = slice(qi * c.CHUNK, (qi + 1) * c.CHUNK)
                        emit_rope(nc, pat, kpel[:, cs], kperaw[:, cs],
                                  cosa_sb[:, cs], sina_sb[:, cs], c.CHUNK)
                    nc.gpsimd.dma_start(out=aglkv_r[:, 0:KT_KV, :], in_=akv_c)
                    nc.gpsimd.dma_start(out=aglkv_r[0:c.DR, KT_KV, :], in_=kpel)
                    if GS > 1:
                        nc.gpsimd.collective_compute(
                            "AllGather", mybir.AluOpType.bypass,
                            replica_groups=groups,
                            ins=[aglkv.ap()], outs=[aggkv.ap()])

                for mti, (seg, ti) in enumerate(mtiles):
                    mw = c.DR if seg == "pe" else 128
                    wa_sb = paw.tile([128, KT_HID, 128], BF, tag="wa_sb",
                                     name="wa_sb")
                    nc.sync.dma_start(
                        out=wa_sb,
                        in_=w_a_r[:, mti].rearrange("p (k m) -> p k m", m=128))
                    if mti == 0:
                        # x tail + small tables behind the first m-tile's
                        # operands
                        for xc in range(2, 4):
                            nc.sync.dma_start(
                                out=x_sb[:, xc * XC:(xc + 1) * XC],
                                in_=xT_r[:, xc * XC:(xc + 1) * XC])
                        nc.sync.dma_start(out=cosa_sb, in_=cosA.ap())
                        nc.sync.dma_start(out=sina_sb, in_=sinA.ap())
                        nc.sync.dma_start(out=lnkv_sb, in_=lnkv.ap())
                    for qi in range(NLC):
                        cs = slice(qi * c.CHUNK, (qi + 1) * c.CHUNK)
                        ps = paps.tile([128, c.CHUNK], F32, tag="aps",
                                       name="ps")
                        for kt in range(KT_HID):
                            nc.tensor.matmul(
                                ps[:mw], wa_sb[:, kt, :mw],
                                x_sb[:, kt, cs],
                                start=(kt == 0), stop=(kt == KT_HID - 1))
                        if seg == "q":
                            nc.scalar.copy(aq8[:, ti, cs], ps)
                            sq = pat.tile([128, c.CHUNK], BF, tag="sq",
                                          bufs=3, name="sq")
                            nc.scalar.square(sq, ps)
                            nc.tensor.matmul(
                                ssq_q[:, cs], ones_c, sq,
                                start=(ti == 0), stop=(ti == KT_Q - 1))
                        elif seg == "kv":
                            nc.scalar.copy(akv_c[:, ti, cs], ps)
                            sq = pat.tile([128, c.CHUNK], BF, tag="sq",
                                          bufs=3, name="sq")
                            nc.scalar.square(sq, ps)
                            nc.tensor.matmul(
                                ssq_kv[:, cs], ones_c, sq,
                                start=(ti == 0), stop=(ti == KT_KV - 1))
                        else:
                            nc.scalar.copy(kperaw[:, cs], ps[:mw])
                    if seg == "pe":
                        emit_kv_part()
                # q-part: the raw latents were evicted as fp8 already;
                # compute the per-token 1/rms, cast it to fp8, and ship it
                # in-band as an extra row of each gathered half-slab.  The
                # consumer (phase C) applies the normalization column-scale.
                aglq_list = [aglq0_r, aglq1_r]
                agl_t = [aglq0, aglq1]
                agg_t = [aggq0, aggq1]
                for hh in (0, 1):
                    cs = slice(hh * HCH, (hh + 1) * HCH)
                    rn = pat.tile([1, HCH], F32, tag="rn", bufs=1, name="rn")
                    nc.scalar.activation(rn, ssq_q[:, cs], Act.Sqrt,
                                         bias=eps_sb, scale=1.0 / c.QLR)
                    rnr = pat.tile([1, HCH], F32, tag="rnr", bufs=1,
                                   name="rnr")
                    nc.vector.reciprocal(rnr, rn)
                    nc.vector.tensor_copy(out=aq8r[:, hh, :], in_=rnr)
                    nc.gpsimd.dma_start(out=aglq_list[hh][:, 0:KT_Q, :],
                                        in_=aq8[:, :, cs])
                    nc.gpsimd.dma_start(out=aglq_list[hh][0:1, KT_Q, :],
                                        in_=aq8r[:, hh, :])
                    if GS > 1:
                        nc.gpsimd.collective_compute(
                            "AllGather", mybir.AluOpType.bypass,
                            replica_groups=groups,
                            ins=[agl_t[hh].ap()], outs=[agg_t[hh].ap()])
                # produce the latent-scale row via Exp, reading the LAST
                # Sqrt's output with scale=0 so the exp-table load is pinned
                # after all Sqrts and before phase C (not at D's first exp)
                nc.scalar.activation(ones_fb, rn[:, 0:128], Act.Exp,
                                     bias=lnsc_sb, scale=0.0)
                # gathered kv latents + shared roped k_pe: gpsimd queue
                # (nothing independent ever sits behind the collective chain
                # there), split into small slices to stay under the SWDGE
                # descriptor-ring capacity
                for g in range(GS):
                    nc.gpsimd.dma_start(
                        out=kpe[:, g * SL:(g + 1) * SL],
                        in_=aggkv_r[g, 0:c.DR, KT_KV, :])
                for g in range(GS):
                    nc.gpsimd.dma_start(
                        out=akv_f[:, :, g * SL:(g + 1) * SL],
                        in_=aggkv_r[g, :, 0:KT_KV, :])

            # ------------- phase B: kv up-projection -------------------------
            if "B" in enabled:
                pkv = top.enter_context(tc.tile_pool(name="pkv", bufs=1))
                knope = pkv.tile([128, H, c.S], BF, tag="knope")
                vv = pkv.tile([128, ST, H, c.DV + 1], BF, tag="vv")
                nc.vector.memset(vv[:, :, :, c.DV:], 1.0)

                with contextlib.ExitStack() as st:
                    pbw = st.enter_context(tc.tile_pool(name="pbw", bufs=1))
                    pbps = st.enter_context(
                        tc.tile_pool(name="pbps", bufs=3, space="PSUM"))
                    wkv_sb = pbw.tile([128, KT_KV, KROWS + VCOLS], BF,
                                      tag="wkv")
                    for kt in range(KT_KV):
                        nc.sync.dma_start(out=wkv_sb[:, kt],
                                          in_=w_kvb_r[:, kt])
                    for mt in range(H):
                        for qc in range(NQC):
                            ps = pbps.tile([128, c.CHUNK], F32, tag="kps")
                            for kt in range(KT_KV):
                                nc.tensor.matmul(
                                    ps, wkv_sb[:, kt, mt * 128:(mt + 1) * 128],
                                    akv_f[:, kt, qc * c.CHUNK:(qc + 1) * c.CHUNK],
                                    start=(kt == 0), stop=(kt == KT_KV - 1))
                            nc.scalar.copy(
                                knope[:, mt, qc * c.CHUNK:(qc + 1) * c.CHUNK], ps)
                    vch = []
                    v0 = 0
                    while v0 < VCOLS:
                        vw = min(512, VCOLS - v0)
                        vch.append((v0, vw))
                        v0 += vw
                    for stt in range(ST):
                        for v0, vw in vch:
                            ps = pbps.tile([128, 512], F32, tag="vps")
                            for kt in range(KT_KV):
                                nc.tensor.matmul(
                                    ps[:, :vw],
                                    akv_f[:, kt, stt * 128:(stt + 1) * 128],
                                    wkv_sb[:, kt, KROWS + v0:KROWS + v0 + vw],
                                    start=(kt == 0), stop=(kt == KT_KV - 1))
                            h0, hn = v0 // c.DV, vw // c.DV
                            nc.scalar.copy(
                                vv[:, stt, h0:h0 + hn, 0:c.DV],
                                ps[:, :vw].rearrange("p (h d) -> p h d", d=c.DV))

            # head-0 q tiles are preloaded into a dedicated pool during C's
            # tail so phase D's first scores don't wait on the qTs round-trip
            pdq0 = top.enter_context(tc.tile_pool(name="pdq0", bufs=1))
            qn0t = pdq0.tile([128, c.S], FP8, tag="qn0t")
            qp0t = pdq0.tile([c.DR, c.S], FP8, tag="qp0t")

            # ------------- phase C: q up-projection + rope + spill -----------
            if "C" in enabled:
                with contextlib.ExitStack() as st:
                    pcw = st.enter_context(tc.tile_pool(name="pcw", bufs=1))
                    pca = st.enter_context(tc.tile_pool(name="pca", bufs=4))
                    pce = st.enter_context(tc.tile_pool(name="pce", bufs=3))
                    pcr = st.enter_context(tc.tile_pool(name="pcr", bufs=3))
                    pcps = st.enter_context(
                        tc.tile_pool(name="pcps", bufs=4, space="PSUM"))
                    wq_sb = pcw.tile([128, MT_QB, KT_Q, 128], FP8, tag="wq")
                    cos_sb = pcw.tile([128, c.S], BF, tag="cos_sb")
                    sin_sb = pcw.tile([128, c.S], BF, tag="sin_sb")
                    nc.sync.dma_start(out=cos_sb, in_=cosT.ap())
                    nc.sync.dma_start(out=sin_sb, in_=sinT.ap())
                    for mt in range(MT_QB):
                        nc.sync.dma_start(out=wq_sb[:, mt], in_=w_qb_r[:, mt])
                    aggq_list = [aggq0_r, aggq1_r]
                    pcbc = st.enter_context(
                        tc.tile_pool(name="pcbc", bufs=2, space="PSUM"))
                    for hh in range(2):
                        asb8s, bcss = [], []
                        with tc.tile_wait_until(0.30, enable=(hh == 1)):
                            for qc in range(NQC):
                                t8 = pca.tile([128, KT_Q + 1, HCH], FP8,
                                              tag="asb8", bufs=3,
                                              name="asb8")
                                nc.gpsimd.dma_start(out=t8,
                                                    in_=aggq_list[hh][qc])
                                asb8s.append(t8)
                        # per-token rms column-scales for all 4 sub-chunks up
                        # front so the Pool pre-scales can run ahead
                        for qc in range(NQC):
                            rnrb = pca.tile([1, HCH], BF, tag="rnrb",
                                            bufs=2, name="rnrb")
                            nc.vector.tensor_copy(
                                out=rnrb, in_=asb8s[qc][0:1, KT_Q, :])
                            bcq = pcbc.tile([128, HCH], F32, tag="bcq",
                                            name="bcq")
                            nc.tensor.matmul(bcq, ones_fb, rnrb,
                                             start=True, stop=True)
                            bcs = pca.tile([128, HCH], BF, tag="bcs",
                                           name="bcs")
                            nc.vector.tensor_copy(out=bcs, in_=bcq)
                            bcss.append(bcs)

                        for qc in range(NQC):
                            col = qc * c.CHUNK + hh * HCH
                            # normalization applied to the latents (Pool) so
                            # the PE consumes plain bf16 and evictions stay on
                            # the Act engine
                            asb16 = pca.tile([128, KT_Q, HCH], BF,
                                             tag="asb16", name="asb16",
                                             bufs=3)
                            for kt in range(KT_Q):
                                nc.gpsimd.tensor_tensor(
                                    out=asb16[:, kt], in0=asb8s[qc][:, kt],
                                    in1=bcss[qc], op=Alu.mult)
                            # pe m-tiles early so their DVE ropes overlap the
                            # later nope m-tiles (and head 0's rows land first)
                            mt_order = [MT_QN, 0, 1, MT_QN + 1, 2, 3,
                                        MT_QN + 2, 4, 5, MT_QN + 3, 6, 7,
                                        MT_QN + 4, 8, 9]
                            for mt in mt_order:
                                m0 = mt * 128
                                ps = pcps.tile([128, HCH], F32, tag="qps")
                                for kt in range(KT_Q):
                                    nc.tensor.matmul(
                                        ps, wq_sb[:, mt, kt, :],
                                        asb16[:, kt, :],
                                        start=(kt == 0), stop=(kt == KT_Q - 1))
                                qsb = pce.tile(
                                    [128, HCH], FP8,
                                    tag="qsbp" if mt >= MT_QN else "qsb")
                                nc.scalar.copy(qsb, ps)
                                if mt >= MT_QN:
                                    roped = pce.tile([128, HCH], FP8,
                                                     tag="roped")
                                    for j in (0, 1):
                                        emit_rope(
                                            nc, pce,
                                            roped[j * 64:(j + 1) * 64],
                                            qsb[j * 64:(j + 1) * 64],
                                            cos_sb[:, col:col + HCH],
                                            sin_sb[:, col:col + HCH], HCH,
                                            p0=j * 64)
                                    qsb = roped
                                nc.sync.dma_start(
                                    out=qTs_ap[m0:m0 + 128, col:col + HCH],
                                    in_=qsb)
                                if hh == 1 and qc == NQC - 1 and mt == 0:
                                    nc.scalar.dma_start(
                                        out=qp0t,
                                        in_=qTs_ap[MT_QN * 128:
                                                   MT_QN * 128 + c.DR, :])
                                    nc.scalar.dma_start(
                                        out=qn0t, in_=qTs_ap[0:128, :])

            # ---------------- phase D: attention -----------------------------
            if "D" in enabled:
                pot = top.enter_context(tc.tile_pool(name="pot", bufs=1))
                oT = pot.tile([128, H, c.S], BF, tag="oT")
                mask_sb = pot.tile([128, TPC, c.CHUNK], FP8, tag="mask_sb")
                nc.sync.dma_start(out=mask_sb, in_=maskm.ap())

                with contextlib.ExitStack() as st:
                    pdp = st.enter_context(tc.tile_pool(name="pdp", bufs=2))
                    pdq = st.enter_context(tc.tile_pool(name="pdq", bufs=2))
                    pde = st.enter_context(tc.tile_pool(name="pde", bufs=6))
                    pds = st.enter_context(
                        tc.tile_pool(name="pds", bufs=3, space="PSUM"))
                    pdo = st.enter_context(
                        tc.tile_pool(name="pdo", bufs=1, space="PSUM"))
                    pdt = st.enter_context(
                        tc.tile_pool(name="pdt", bufs=1, space="PSUM"))
                    for h in range(H):
                        if h == 0:
                            qn, qp = qn0t, qp0t
                        else:
                            qn = pdq.tile([128, c.S], FP8, tag="qn")
                            nc.scalar.dma_start(
                                out=qn, in_=qTs_ap[h * 128:(h + 1) * 128, :])
                            qp = pdq.tile([c.DR, c.S], FP8, tag="qp")
                            r0 = MT_QN * 128 + h * c.DR
                            nc.scalar.dma_start(out=qp,
                                                in_=qTs_ap[r0:r0 + c.DR, :])
                        for qc in range(NQC):
                            col = qc * c.CHUNK
                            nfull = TPC * qc
                            probs = (pdp.tile([128, ST - TPC, c.CHUNK], BF,
                                              tag="probs", name="probs")
                                     if nfull else None)
                            probsD = pdp.tile([128, 1280], BF, tag="probsD")
                            # full (unmasked) key tiles, exp fused in pairs
                            for j in range(nfull // 2):
                                ps = pds.tile([128, 2 * c.CHUNK], F32,
                                              tag="sc", name="sc")
                                for i in (0, 1):
                                    kt = 2 * j + i
                                    sl2 = slice(i * c.CHUNK, (i + 1) * c.CHUNK)
                                    nc.tensor.matmul(
                                        ps[:, sl2],
                                        knope[:, h, kt * 128:(kt + 1) * 128],
                                        qn[:, col:col + c.CHUNK],
                                        start=True, stop=False)
                                    nc.tensor.matmul(
                                        ps[:, sl2],
                                        kpe[:, kt * 128:(kt + 1) * 128],
                                        qp[:, col:col + c.CHUNK],
                                        start=False, stop=True)
                                nc.scalar.activation(
                                    probs[:, 2 * j:2 * j + 2, :], ps, Act.Exp,
                                    scale=ESC)
                            # diagonal tiles at 128-query granularity, packed
                            # pairwise into PSUM so exp runs in 2 calls
                            DOFF = (0, 512, 896, 1152)
                            for dpair in ((0, 1), (2, 3)):
                                ps = pds.tile([128, 2 * c.CHUNK], F32,
                                              tag="sc", name="sc")
                                base = DOFF[dpair[0]]
                                for d in dpair:
                                    kt = nfull + d
                                    w = c.CHUNK - 128 * d
                                    o = DOFF[d] - base
                                    nc.tensor.matmul(
                                        ps[:, o:o + w],
                                        knope[:, h, kt * 128:(kt + 1) * 128],
                                        qn[:, col + 128 * d:col + c.CHUNK],
                                        start=True, stop=False)
                                    nc.tensor.matmul(
                                        ps[:, o:o + w],
                                        kpe[:, kt * 128:(kt + 1) * 128],
                                        qp[:, col + 128 * d:col + c.CHUNK],
                                        start=False, stop=True)
                                wtot = (DOFF[dpair[1]] - base
                                        + c.CHUNK - 128 * dpair[1])
                                nc.scalar.activation(
                                    probsD[:, base:base + wtot],
                                    ps[:, :wtot], Act.Exp, scale=ESC)
                            for d in range(TPC):
                                w = c.CHUNK - 128 * d
                                nc.vector.tensor_tensor(
                                    out=probsD[:, DOFF[d]:DOFF[d] + w],
                                    in0=probsD[:, DOFF[d]:DOFF[d] + w],
                                    in1=mask_sb[:, d, 128 * d:], op=Alu.mult)
                            for q2 in range(TPC):
                                qt = TPC * qc + q2
                                po = pdo.tile([128, c.DV + 1], F32, tag="po")
                                for kt in range(qt + 1):
                                    if kt < nfull:
                                        lh = probs[:, kt, q2 * 128:(q2 + 1) * 128]
                                    else:
                                        d = kt - nfull
                                        lh = probsD[:, DOFF[d] + (q2 - d) * 128:
                                                    DOFF[d] + (q2 - d) * 128 + 128]
                                    nc.tensor.matmul(
                                        po, lh, vv[:, kt, h, :],
                                        start=(kt == 0), stop=(kt == qt))
                                rec = pde.tile([128, 1], F32, tag="rec")
                                nc.vector.reciprocal(rec, po[:, c.DV:c.DV + 1])
                                osb = pde.tile([128, c.DV], BF, tag="osb")
                                nc.vector.tensor_scalar_mul(
                                    osb, po[:, :c.DV], rec)
                                pt = pdt.tile([128, 128], BF, tag="pt")
                                nc.tensor.transpose(pt, osb, ident)
                                nc.vector.tensor_copy(
                                    out=oT[:, h, qt * 128:(qt + 1) * 128],
                                    in_=pt)

            # ---------------- phase E: o-projection --------------------------
            if "E" in enabled:
                with contextlib.ExitStack() as st:
                    pew = st.enter_context(tc.tile_pool(name="pew", bufs=3))
                    peo = st.enter_context(tc.tile_pool(name="peo", bufs=3))
                    peps = st.enter_context(
                        tc.tile_pool(name="peps", bufs=4, space="PSUM"))
                    for mt in range(MT_O):
                        wo_sb = pew.tile([128, H, 128], BF, tag="wo")
                        nc.sync.dma_start(
                            out=wo_sb,
                            in_=w_o_r[:, mt].rearrange(
                                "p (k m) -> p k m", m=128))
                        for qc in range(NQC):
                            col = qc * c.CHUNK
                            ps = peps.tile([128, c.CHUNK], F32, tag="ops")
                            for kt in range(H):
                                nc.tensor.matmul(
                                    ps, wo_sb[:, kt, :],
                                    oT[:, kt, col:col + c.CHUNK],
                                    start=(kt == 0), stop=(kt == H - 1))
                            ob = peo.tile([128, c.CHUNK], F32, tag="ob")
                            nc.scalar.copy(ob, ps)
                            nc.sync.dma_start(
                                out=outT_ap[mt * 128:(mt + 1) * 128,
                                            col:col + c.CHUNK],
                                in_=ob)

    nc.compile()
    return nc


# ---------------------------------------------------------------------------
# host-side input preparation
# ---------------------------------------------------------------------------

def prep_shared(c: Cfg, w_a, q_ln_w, kv_ln_w):
    KT_Q = c.QLR // 128
    KT_KV = c.KVLR // 128
    TPC = c.CHUNK // 128
    half = c.PEH
    inv_freq = 1.0 / (c.THETA ** (np.arange(half, dtype=np.float32) / half))
    ang = np.arange(c.S, dtype=np.float32)[:, None] * inv_freq[None, :]
    cosT = np.ascontiguousarray(
        np.tile(np.cos(ang).T, (128 // half, 1))).astype(BF16)
    sinT = np.ascontiguousarray(
        np.tile(np.sin(ang).T, (128 // half, 1))).astype(BF16)
    k_idx = np.arange(128)[:, None]
    q_idx = np.arange(c.CHUNK)[None, :]
    maskm = np.stack(
        [(k_idx <= q_idx - 128 * d) for d in range(TPC)], axis=1
    ).astype(ml_dtypes.float8_e3m4)
    # w_a tiled: [p, mt, kt, 128] with the pe m-tile zero-padded to 128 cols
    MT_A = KT_Q + KT_KV + 1
    KT_HID = c.HID // 128
    wa = np.asarray(w_a, np.float32)
    # m-tile order in the kernel: kv tiles, pe, then q tiles
    order = ([c.QLR + i * 128 for i in range(KT_KV)]
             + [c.QLR + c.KVLR]
             + [i * 128 for i in range(KT_Q)])
    tiles = []
    for m0 in order:
        t = np.zeros((c.HID, 128), np.float32)
        wsrc = wa[:, m0:m0 + 128]
        t[:, :wsrc.shape[1]] = wsrc
        tiles.append(t)
    wa_t = np.stack(tiles, axis=1)  # [HID, MT_A, 128]
    wa_t = wa_t.reshape(KT_HID, 128, MT_A, 128).transpose(1, 2, 0, 3)
    wa_t = np.ascontiguousarray(wa_t.reshape(128, MT_A * KT_HID * 128))
    return {
        "w_a": wa_t.astype(BF16),
        "lnkv": np.ascontiguousarray(
            kv_ln_w.reshape(KT_KV, 128).T).astype(np.float32),
        "cosT": cosT,
        "sinT": sinT,
        "maskm": np.ascontiguousarray(maskm),
    }


def prep_group(c: Cfg, heads, w_qb, w_kvb, w_o, n_heads_total):
    """Reorganize the up-projection weights for one head group."""
    wq = w_qb.reshape(c.QLR, n_heads_total, c.DQK)[:, heads, :]
    wq_g = np.concatenate(
        [wq[:, :, :c.DN].reshape(c.QLR, -1), wq[:, :, c.DN:].reshape(c.QLR, -1)],
        axis=1)
    wkv = w_kvb.reshape(c.KVLR, n_heads_total, c.DN + c.DV)[:, heads, :]
    wkv_g = np.concatenate(
        [wkv[:, :, :c.DN].reshape(c.KVLR, -1),
         wkv[:, :, c.DN:].reshape(c.KVLR, -1)], axis=1)
    wo_g = w_o.reshape(n_heads_total, c.DV, c.HID)[heads].reshape(-1, c.HID)
    H = c.HPC
    KT_Q = c.QLR // 128
    KT_KV = c.KVLR // 128
    MT_QB = wq_g.shape[1] // 128
    MT_O = c.HID // 128
    # device layouts: [p, mt, kt, 128] flattened per partition
    wq_t = wq_g.reshape(KT_Q, 128, MT_QB, 128).transpose(1, 2, 0, 3)
    wq_t = np.ascontiguousarray(wq_t.reshape(128, MT_QB * KT_Q * 128))
    wkv_t = wkv_g.reshape(KT_KV, 128, wkv_g.shape[1]).transpose(1, 0, 2)
    wkv_t = np.ascontiguousarray(wkv_t.reshape(128, -1))
    wo_t = wo_g.reshape(H, 128, MT_O, 128).transpose(1, 2, 0, 3)
    wo_t = np.ascontiguousarray(wo_t.reshape(128, MT_O * H * 128))
    return {
        "w_qb": (wq_t * 32.0).astype(ml_dtypes.float8_e3m4),
        "w_kvb": wkv_t.astype(BF16),
        "w_o": wo_t.astype(BF16),
    }


_PROGRAM = None


def _get_program():
    global _PROGRAM
    if _PROGRAM is None:
        _PROGRAM = build_program(FULL)
    return _PROGRAM


def kernel(x, w_a, q_ln_w, kv_ln_w, w_qb, w_kvb, w_o):
    from concourse.bass_utils import run_bass_kernel_spmd

    c = FULL
    x = np.asarray(x, dtype=np.float32)
    B = x.shape[0]
    n_heads = w_qb.shape[1] // c.DQK
    n_groups = n_heads // c.HPC
    assert B * n_groups == c.NCORES and n_groups == c.GS

    nc = _get_program()
    shared = prep_shared(c, np.asarray(w_a), np.asarray(q_ln_w),
                         np.asarray(kv_ln_w))
    # q rmsnorm weight is folded into the q up-projection rows (the kernel
    # ships only the per-token 1/rms scale)
    w_qb_eff = np.asarray(w_qb, np.float32) * np.asarray(
        q_ln_w, np.float32)[:, None]
    groups = [
        prep_group(c, slice(g * c.HPC, (g + 1) * c.HPC), w_qb_eff,
                   np.asarray(w_kvb), np.asarray(w_o), n_heads)
        for g in range(n_groups)
    ]
    xTs = [np.ascontiguousarray(x[b].T).astype(BF16) for b in range(B)]

    in_maps = []
    for core in range(c.NCORES):
        b, g = divmod(core, n_groups)
        sl = slice(g * c.SL, (g + 1) * c.SL)
        xtl = xTs[b][:, sl].reshape(c.HID // 128, 128, c.SL)
        xtl = np.ascontiguousarray(
            xtl.transpose(1, 0, 2).reshape(128, -1))
        in_maps.append({
            "xT": xtl,
            "cosA": np.ascontiguousarray(shared["cosT"][:, sl]),
            "sinA": np.ascontiguousarray(shared["sinT"][:, sl]),
            **shared, **groups[g],
        })

    res = run_bass_kernel_spmd(nc, in_maps, core_ids=list(range(c.NCORES)))
    outs = [r["outT"] for r in res.results]
    result = np.empty((B, c.S, c.HID), dtype=np.float32)
    for b in range(B):
        acc = outs[b * n_groups].copy()
        for g in range(1, n_groups):
            acc += outs[b * n_groups + g]
        result[b] = acc.T
    return result
